# revision 32
# baseline (speedup 1.0000x reference)
"""Single-head dense attention (B=4, S=2048, H=1024) on 8 TRN2 NeuronCores.

Sharding: data-parallel, core c -> (batch b = c//2, query-half h = c%2).
Each core receives its batch's x in two layouts (xT [H, S] for scores-rhs and
x [S, H] for the P@x contraction), both fp16 and rolled so the core's 1024
queries are always keys 0:1024 (attention is key-order invariant), plus the
four weights fp16 (Wq/Wk/Wv in original [out, in] layout, Wo transposed).

Math is reassociated to cut matmul FLOPs ~20% vs the direct form:
  S   = Q K^T = x_q (Wq^T Wk) x^T     -> Wqk = Wq^T Wk, A = x_q Wqk, S = A x^T
  y   = P V Wo^T = P x (Wv^T Wo^T)    -> Wvo = Wv^T Wo^T, T = P x, y = T Wvo
so K, Q, V, out are never materialized; the projections become two
weight-weight products (1024^3 each) plus A (1024^3), replacing
K/Q/V-proj + PV + y-proj (note V-proj's 2048-row redundancy is gone).

Wvo is additionally split 8 ways: each core computes one 128-row e'-tile
(its Wv column slice arrives pre-sliced from the host) and an AllGather
assembles the full product during the score phase, long before the y phase
reads it — cutting another 112 of 128 Wvo matmuls per core.

Per-core pipeline (fp16 matmuls, fp32 PSUM accumulation, weight-stationary
loop order so each LDWEIGHTS is amortized over the matmuls sharing its lhsT):
  1. Wqk via two waves of 4 open PSUM accumulation groups (ec-outer, so PE
     consumes each wq/wk chunk pair as its DMA lands); A^T[d',q] = Wqk^T x^T.
  2. Scores per 128-query chunk in natural [q, ks] layout (lhsT = A^T chunk
     amortized over 4 key tiles); per-row max (DVE reduce_max on the score
     PSUM) as exp bias, exp to fp16, denominators free via ACT accum_out;
     the exp tile is PE-transposed UNnormalized (identity rhs) so the
     transpose waits only on exp, and 1/denom is applied later at the y
     drain where queries are back on the partition axis.  The Wvo slice
     chain + AllGather sit at qc0 as softmax-latency filler.
  3. T^T[e,q] = x^T exp^T with each x-chunk lhsT shared across both q-tiles;
     y = T^T.T @ Wvo row-scaled by 1/denom during the drain; the last query
     tile is split into two half-chains so its drain+DMA overlaps compute.
"""

from contextlib import ExitStack

import numpy as np

import bass_rust
import concourse.bass as bass
import concourse.mybir as mybir
import concourse.tile as tile
from concourse.vector_clock import ScopedClock
from concourse.masks import make_identity

HIDDEN = 1024
BATCH, SEQ = 4, 2048
P = 128
QH = 1024  # queries per core
NCORES = 8

F16 = mybir.dt.float16
F32 = mybir.dt.float32

# ---------------------------------------------------------------------------
# Workaround: walrus in this container encodes a limited number of sync-wait
# commands per instruction (1 for Matmult/Ldweights, ~4 for control insts).
# Split overflow waits onto same-engine NOPs inserted before the instruction,
# and split the Tile tail-drain waits onto sync-engine NOPs.
# ---------------------------------------------------------------------------
_MAXW = 1
_nop_ctr = [0]


def _patched_drain_and_barrier(self, tick_clock, wait_clock):
    nc = self.nc
    drain_inst = nc.sync.drain()
    wait_clock.add_sem_waits(
        drain_inst.ins, ScopedClock({None: tick_clock.global_clock})
    )
    si = drain_inst.ins.sync_info
    if si is not None and len(si.on_wait) > _MAXW:
        waits = list(si.on_wait)
        drain_inst.ins.sync_info = bass_rust.SyncInfo(
            on_wait=waits[:_MAXW], on_update=list(si.on_update)
        )
        for i in range(_MAXW, len(waits), _MAXW):
            nop = nc.sync.nop()
            nop.ins.sync_info = bass_rust.SyncInfo(
                on_wait=waits[i : i + _MAXW], on_update=[]
            )
    nc.all_engine_barrier()
    assert self.sems is not None
    popped = nc._tile_sem_poison_stack.pop()
    assert popped is self._sem_poison
    nc.clear_and_free_semaphores(list(self.sems.allocated().values()))
    nc.all_engine_barrier()


tile.TileContext._drain_and_barrier = _patched_drain_and_barrier


def _split_all_instruction_waits(nc):
    for f in nc.m.functions:
        for b in f.blocks:
            insts = b.instructions
            new_list = []
            changed = False
            for inst in insts:
                si = getattr(inst, "sync_info", None)
                if si is not None and len(si.on_wait) > _MAXW:
                    waits = list(si.on_wait)
                    keep = waits[-_MAXW:]
                    overflow = waits[:-_MAXW]
                    for j in range(0, len(overflow), _MAXW):
                        _nop_ctr[0] += 1
                        nop = mybir.InstNoOp(
                            name=f"I-waitsplit-{_nop_ctr[0]}",
                            engine=inst.engine,
                            bass_nofuse=True,
                            sync_info=bass_rust.SyncInfo(
                                on_wait=overflow[j : j + _MAXW], on_update=[]
                            ),
                        )
                        new_list.append(nop)
                    inst.sync_info = bass_rust.SyncInfo(
                        on_wait=keep, on_update=list(si.on_update)
                    )
                    changed = True
                new_list.append(inst)
            if changed:
                insts[:] = new_list


# ---------------------------------------------------------------------------
# Kernel program
# ---------------------------------------------------------------------------

def _build_program(repeat=1, tweak=0):
    nc = bass.Bass(
        "TRN2", target_bir_lowering=False, debug=False, num_devices=NCORES
    )
    xt_d = nc.dram_tensor("xt", [HIDDEN, SEQ], F16, kind="ExternalInput").ap()
    xn_d = nc.dram_tensor("xn", [SEQ, HIDDEN], F16, kind="ExternalInput").ap()
    wq_d = nc.dram_tensor("wq", [HIDDEN, HIDDEN], F16, kind="ExternalInput").ap()
    wk_d = nc.dram_tensor("wk", [HIDDEN, HIDDEN], F16, kind="ExternalInput").ap()
    # this core's 128-column slice of Wv: the Wvo product is computed
    # 1/8th per core and assembled with an AllGather
    wvs_d = nc.dram_tensor("wvs", [HIDDEN, P], F16, kind="ExternalInput").ap()
    wot_d = nc.dram_tensor("wot", [HIDDEN, HIDDEN], F16, kind="ExternalInput").ap()
    y_d = nc.dram_tensor("y", [QH, HIDDEN], F32, kind="ExternalOutput").ap()

    HO = HIDDEN // P  # 8 chunks of the hidden/feature dim
    SO = SEQ // P  # 16 chunks of the key seq dim

    with tile.TileContext(nc) as tc:
      for _t in range(tweak):
        nc.sync.nop()
      for _rep in range(repeat):
        vo_in_d = nc.dram_tensor(f"vo_in_{_rep}", [P, HIDDEN], F16)
        vo_out_d = nc.dram_tensor(
            f"vo_out_{_rep}", [HIDDEN, HIDDEN], F16, addr_space="Shared"
        )
        with ExitStack() as ctx:
            _kernel_body(nc, tc, ctx, xt_d, xn_d, wq_d, wk_d, wvs_d, wot_d,
                         vo_in_d, vo_out_d, y_d, HO, SO)

    _split_all_instruction_waits(nc)
    return nc


def _kernel_body(nc, tc, ctx, xt_d, xn_d, wq_d, wk_d, wvs_d, wot_d,
                 vo_in_d, vo_out_d, y_d, HO, SO):
    if True:
        pers = ctx.enter_context(tc.tile_pool(name="pers", bufs=1))
        xt_sb = pers.tile([P, HO, SEQ], F16)  # x^T [d, ks] (queries = 0:QH)
        xn_sb = pers.tile([P, SO, HIDDEN], F16)  # x [ks, e]
        wvo_sb = pers.tile([P, HO, HIDDEN], F16)  # Wvo [e', f] (via AllGather)
        at_sb = pers.tile([P, HO, QH], F16)  # A^T [d', q]
        wvs_sb = pers.tile([P, HO, P], F16)  # Wv [e, e'-slice] (this core's)
        wot_sb = pers.tile([P, HO, HIDDEN], F16)  # Wo^T [e, f]

        # ---- Phase 1: Wqk (full, two 4-group waves) + A^T; the Wvo slice
        # chain + AllGather live in the score phase as softmax filler ----
        with ExitStack() as p1:
            proj = p1.enter_context(tc.tile_pool(name="proj", bufs=1))
            wq_sb = proj.tile([P, HO, HIDDEN], F16, tag="wq")
            wk_sb = proj.tile([P, HO, HIDDEN], F16, tag="wk")
            wqk_sb = proj.tile([P, HO, HIDDEN], F16, tag="wqk")  # Wqk [d, d']
            xt_r = xt_d.rearrange("(o p) s -> p o s", p=P)
            xn_r = xn_d.rearrange("(o p) e -> p o e", p=P)
            wq_r = wq_d.rearrange("(o p) d -> p o d", p=P)
            wk_r = wk_d.rearrange("(o p) d -> p o d", p=P)
            wvs_r = wvs_d.rearrange("(o p) j -> p o j", p=P)
            wot_r = wot_d.rearrange("(o p) f -> p o f", p=P)
            # Interleave wq/wk chunk DMAs so the Wqk wave consumes each pair
            # as it lands; xt next (A^T + scores), wvs/wot (Wvo slice chain
            # in the score phase), then xn (phase 3).
            nc.sync.dma_start(wq_sb[:, 0, 0:512], wq_r[:, 0, 0:512])
            nc.sync.dma_start(wk_sb[:, 0, 0:512], wk_r[:, 0, 0:512])
            nc.sync.dma_start(wk_sb[:, 0, 512:1024], wk_r[:, 0, 512:1024])
            nc.sync.dma_start(wq_sb[:, 0, 512:1024], wq_r[:, 0, 512:1024])
            for dc in range(1, HO):
                nc.sync.dma_start(wq_sb[:, dc], wq_r[:, dc])
                nc.sync.dma_start(wk_sb[:, dc], wk_r[:, dc])
            for dc in range(HO):
                nc.sync.dma_start(xt_sb[:, dc], xt_r[:, dc])
            nc.sync.dma_start(wvs_sb[:], wvs_r[:])
            for dc in range(HO):
                nc.sync.dma_start(wot_sb[:, dc], wot_r[:, dc])
            for sc in range(SO):
                nc.sync.dma_start(xn_sb[:, sc], xn_r[:, sc])

            ps_proj = p1.enter_context(
                tc.tile_pool(name="ps_proj", bufs=1, space="PSUM")
            )
            # Wqk[d, d'] = sum_e Wq[e, d] Wk[e, d'].  Two waves of 4 open
            # accumulation groups (8 PSUM banks), ec-outer: PE consumes each
            # wq/wk chunk pair as its DMA lands instead of stalling on the
            # full 4MB during the first output tile.
            for w in range(2):
                pts = [
                    ps_proj.tile([P, 2, 512], F32, tag=f"pj{g}", name=f"qk{g}")
                    for g in range(4)
                ]
                for ec in range(HO):
                    for g in range(4):
                        dt = w * 4 + g
                        for h in range(2):
                            nc.tensor.matmul(
                                pts[g][:, h],
                                lhsT=wq_sb[:, ec, dt * P : (dt + 1) * P],
                                rhs=wk_sb[:, ec, h * 512 : (h + 1) * 512],
                                start=(ec == 0),
                                stop=(ec == HO - 1),
                            )
                for g in range(4):
                    nc.vector.tensor_copy(out=wqk_sb[:, w * 4 + g], in_=pts[g][:])
            # A^T[d', q] = sum_d Wqk[d, d'] xT[d, q]
            for dt in range(HO):
                pt = ps_proj.tile(
                    [P, 2, 512], F32, tag=f"pj{dt % 2}", name="pt_at"
                )
                for dc in range(HO):
                    for h in range(2):
                        nc.tensor.matmul(
                            pt[:, h],
                            lhsT=wqk_sb[:, dc, dt * P : (dt + 1) * P],
                            rhs=xt_sb[:, dc, h * 512 : (h + 1) * 512],
                            start=(dc == 0),
                            stop=(dc == HO - 1),
                        )
                if dt % 2 == 0:
                    nc.vector.tensor_copy(out=at_sb[:, dt], in_=pt[:])
                else:
                    nc.scalar.copy(out=at_sb[:, dt], in_=pt[:])

        # ---- Phase 2: scores + softmax + fused transpose (unnormalized:
        # P^T = exp^T; the 1/denominator is folded into the y drain, where
        # queries are back on the partition axis) ----
        pers2 = ctx.enter_context(tc.tile_pool(name="pers2", bufs=1))
        phat_sb = pers2.tile([P, SO, QH], F16)  # exp^T [ks, q]
        tt_sb = pers2.tile([P, HO, QH], F16)  # T^T [e, q] (unnormalized)
        ident = pers2.tile([P, P], F16)
        make_identity(nc, ident[:])
        attb = ctx.enter_context(tc.tile_pool(name="attb", bufs=3))
        smallb = ctx.enter_context(tc.tile_pool(name="smallb", bufs=6))
        recs = ctx.enter_context(tc.tile_pool(name="recs", bufs=QH // P))
        ystage = ctx.enter_context(tc.tile_pool(name="ystage", bufs=2))
        ps_sc = ctx.enter_context(tc.tile_pool(name="ps_sc", bufs=3, space="PSUM"))
        ps_tp = ctx.enter_context(tc.tile_pool(name="ps_tp", bufs=2, space="PSUM"))
        rec_tiles = []

        NQ = QH // P  # 8 query chunks of 128
        for qc in range(NQ):
            exp_sb = attb.tile([P, 4, 512], F16, tag="expP", name="exp_sb")
            sc_tiles = [
                ps_sc.tile([P, 2, 512], F32, tag="sc", name="sc_ps")
                for _ in range(2)
            ]
            # S[q, ks] = sum_d' A[q, d'] x[ks, d']; one lhsT load per d'-chunk
            # serves all 4 key tiles.
            for ec in range(HO):
                for kst in range(4):
                    nc.tensor.matmul(
                        sc_tiles[kst // 2][:, kst % 2],
                        lhsT=at_sb[:, ec, qc * P : (qc + 1) * P],
                        rhs=xt_sb[:, ec, kst * 512 : (kst + 1) * 512],
                        start=(ec == 0),
                        stop=(ec == HO - 1),
                    )
            if qc == 0:
                # This core's 1/8 slice of Wvo[e', f] = sum_e Wv[e, e']
                # WoT[e, f]; doubles as PE filler while qc0's softmax runs on
                # DVE/ACT.  The AllGather assembles the full Wvo across the 8
                # cores (each contributes a 128-row e'-tile) well before the
                # y phase needs it.
                vo = ps_sc.tile([P, 2, 512], F32, tag="sc", name="vo_ps")
                for ec in range(HO):
                    for h in range(2):
                        nc.tensor.matmul(
                            vo[:, h],
                            lhsT=wvs_sb[:, ec],
                            rhs=wot_sb[:, ec, h * 512 : (h + 1) * 512],
                            start=(ec == 0),
                            stop=(ec == HO - 1),
                        )
                vo_sl = attb.tile([P, HIDDEN], F16, tag="voslice", name="vo_sl")
                nc.scalar.copy(out=vo_sl[:], in_=vo[:])
                nc.scalar.dma_start(vo_in_d.ap(), vo_sl[:])
                nc.gpsimd.collective_compute(
                    "AllGather",
                    mybir.AluOpType.bypass,
                    replica_groups=[list(range(NCORES))],
                    ins=[vo_in_d.ap().opt()],
                    outs=[vo_out_d.ap().opt()],
                )
                nc.sync.dma_start(
                    wvo_sb[:], vo_out_d.ap().rearrange("(o p) f -> p o f", p=P)
                )
            # exact per-row max -> negated bias for exp
            m0 = smallb.tile([P, 1], F32, tag="m0", name="m0")
            m1 = smallb.tile([P, 1], F32, tag="m1", name="m1")
            nc.vector.reduce_max(m0[:], sc_tiles[0][:], axis=mybir.AxisListType.XY)
            nc.vector.reduce_max(m1[:], sc_tiles[1][:], axis=mybir.AxisListType.XY)
            negmax = smallb.tile([P, 1], F32, tag="negmax", name="negmax")
            nc.vector.tensor_tensor(
                negmax[:], m0[:], m1[:], mybir.AluOpType.max
            )
            nc.vector.tensor_scalar_mul(negmax[:], negmax[:], -1.0)
            accs = []
            for kh in range(2):
                for kst in range(2):
                    acc = smallb.tile([P, 1], F32, tag="acc", name="acc")
                    nc.scalar.activation(
                        exp_sb[:, kh * 2 + kst],
                        sc_tiles[kh][:, kst],
                        mybir.ActivationFunctionType.Exp,
                        bias=negmax[:],
                        scale=1.0,
                        accum_out=acc[:],
                    )
                    accs.append(acc)
            den = smallb.tile([P, 1], F32, tag="den", name="den")
            nc.vector.tensor_add(out=den[:], in0=accs[0][:], in1=accs[1][:])
            nc.vector.tensor_add(out=den[:], in0=den[:], in1=accs[2][:])
            nc.vector.tensor_add(out=den[:], in0=den[:], in1=accs[3][:])
            rec = recs.tile([P, 1], F32, tag="rec", name="rec")
            rec_tiles.append(rec)
            nc.vector.reciprocal(rec[:], den[:])
            # transpose in one regular matmul per 128x128 tile:
            # exp^T[ks, q] = sum_q' exp[q', ks] * I[q', q]
            for half in range(4):
                tp_ps = ps_tp.tile([P, 4, P], F32, tag="tp", name="tp_ps")
                for k4 in range(4):
                    ksc = half * 4 + k4
                    nc.tensor.matmul(
                        tp_ps[:, k4],
                        lhsT=exp_sb[:, ksc // 4, (ksc % 4) * P : (ksc % 4 + 1) * P],
                        rhs=ident[:],
                        start=(k4 == 0),
                        stop=(k4 == 3),
                    )
                nc.vector.tensor_copy(
                    out=phat_sb[:, half * 4 : (half + 1) * 4, qc * P : (qc + 1) * P],
                    in_=tp_ps[:],
                )
        # ---- Phase 3: T^T[e, q] = sum_ks x[ks, e] P^T[ks, q]; both q-tiles
        # share each x-chunk weight load; PSUM slots reuse the score tag.
        for half in range(4):
            pv_ps0 = ps_sc.tile([P, 2, 512], F32, tag="sc", name="pv_ps0")
            pv_ps1 = ps_sc.tile([P, 2, 512], F32, tag="sc", name="pv_ps1")
            for ks in range(SO):
                for e2 in range(2):
                    ec = half * 2 + e2
                    for qt, pv_ps in ((0, pv_ps0), (1, pv_ps1)):
                        nc.tensor.matmul(
                            pv_ps[:, e2],
                            lhsT=xn_sb[:, ks, ec * P : (ec + 1) * P],
                            rhs=phat_sb[:, ks, qt * 512 : (qt + 1) * 512],
                            start=(ks == 0),
                            stop=(ks == SO - 1),
                        )
            for e2 in range(2):
                nc.vector.tensor_copy(
                    out=tt_sb[:, half * 2 + e2, 0:512], in_=pv_ps0[:, e2]
                )
                nc.scalar.copy(
                    out=tt_sb[:, half * 2 + e2, 512:1024], in_=pv_ps1[:, e2]
                )
        # ---- Phase 4: y[sq, f] = sum_e T^T[e, sq] Wvo[e, f]; both f-tiles
        # share each T^T weight load; the softmax 1/denominator is applied
        # here as a per-partition scale during the PSUM drain.  The last
        # query tile is split into two half-chains so its drain + output DMA
        # overlaps the second half's matmuls instead of sitting in the tail.
        for sqc in range(QH // P):
            sq0 = sqc * P
            rec = rec_tiles[sqc]
            last = sqc == QH // P - 1
            if not last:
                y_ps = ps_sc.tile([P, 2, 512], F32, tag="sc", name="y_ps")
                for ec in range(HO):
                    for ft in range(HIDDEN // 512):
                        nc.tensor.matmul(
                            y_ps[:, ft],
                            lhsT=tt_sb[:, ec, sq0 : sq0 + P],
                            rhs=wvo_sb[:, ec, ft * 512 : (ft + 1) * 512],
                            start=(ec == 0),
                            stop=(ec == HO - 1),
                        )
                y_sb = ystage.tile([P, 2, 512], F32, tag="ystage", name="y_sb")
                nc.vector.tensor_scalar_mul(y_sb[:, 0], y_ps[:, 0], rec[:])
                nc.scalar.activation(
                    y_sb[:, 1],
                    y_ps[:, 1],
                    mybir.ActivationFunctionType.Copy,
                    scale=rec[:],
                )
                nc.sync.dma_start(y_d[sq0 : sq0 + P, 0:512], y_sb[:, 0])
                nc.sync.dma_start(y_d[sq0 : sq0 + P, 512:1024], y_sb[:, 1])
            else:
                for ft in range(HIDDEN // 512):
                    y_ph = ps_tp.tile([P, 1, 512], F32, tag="tp", name="y_ph")
                    for ec in range(HO):
                        nc.tensor.matmul(
                            y_ph[:, 0],
                            lhsT=tt_sb[:, ec, sq0 : sq0 + P],
                            rhs=wvo_sb[:, ec, ft * 512 : (ft + 1) * 512],
                            start=(ec == 0),
                            stop=(ec == HO - 1),
                        )
                    y_sb = ystage.tile(
                        [P, 1, 512], F32, tag="ystageh", name="y_sbh"
                    )
                    for qtr in range(2):
                        qs = slice(qtr * 256, (qtr + 1) * 256)
                        if (ft + qtr) % 2 == 0:
                            nc.vector.tensor_scalar_mul(
                                y_sb[:, 0, qs], y_ph[:, 0, qs], rec[:]
                            )
                        else:
                            nc.scalar.activation(
                                y_sb[:, 0, qs],
                                y_ph[:, 0, qs],
                                mybir.ActivationFunctionType.Copy,
                                scale=rec[:],
                            )
                        nc.sync.dma_start(
                            y_d[sq0 : sq0 + P,
                                ft * 512 + qtr * 256 : ft * 512 + (qtr + 1) * 256],
                            y_sb[:, 0, qs],
                        )


_cached_nc = None


def prepare_in_maps(x, Wq, Wk, Wv, Wo):
    x = np.asarray(x, dtype=np.float32)
    Wq = np.asarray(Wq, dtype=np.float32)
    Wk = np.asarray(Wk, dtype=np.float32)
    Wv = np.asarray(Wv, dtype=np.float32)
    Wo = np.asarray(Wo, dtype=np.float32)

    # Host-side layout prep (no FLOPs): cast fp16; Wq/Wk/Wv stay [out, in],
    # Wo is transposed to [in, out].  Core c gets only its 128-column slice
    # of Wv (it computes 1/8 of Wvo; an AllGather assembles the rest).
    wq = Wq.astype(np.float16)
    wk = Wk.astype(np.float16)
    wv = Wv.astype(np.float16)
    wot = np.ascontiguousarray(Wo.T).astype(np.float16)

    in_maps = []
    for c in range(NCORES):
        b, h = divmod(c, 2)
        qlo = h * QH
        # roll keys so this core's queries are always keys 0:QH
        xr = np.concatenate([x[b][qlo:], x[b][:qlo]], axis=0)
        xn = xr.astype(np.float16)
        xt = np.ascontiguousarray(xr.T).astype(np.float16)
        wvs = np.ascontiguousarray(wv[:, c * P : (c + 1) * P])
        in_maps.append(
            {"xt": xt, "xn": xn, "wq": wq, "wk": wk, "wvs": wvs, "wot": wot}
        )
    return in_maps


def kernel(x, Wq, Wk, Wv, Wo):
    global _cached_nc
    from concourse.bass_utils import run_bass_kernel_spmd

    in_maps = prepare_in_maps(x, Wq, Wk, Wv, Wo)

    if _cached_nc is None:
        _cached_nc = _build_program()
    res = run_bass_kernel_spmd(_cached_nc, in_maps, core_ids=list(range(NCORES)))

    out = np.empty((BATCH, SEQ, HIDDEN), dtype=np.float32)
    for c in range(NCORES):
        b, h = divmod(c, 2)
        out[b, h * QH : (h + 1) * QH, :] = res.results[c]["y"]
    return out
